# revision 74
# baseline (speedup 1.0000x reference)
"""Trainium2 Bass kernel for MultiHeadAttentionRoPE.

Problem (hardcoded): B=2, S=2048, D=1024, H=16 heads, Dh=64, fp32 in/out.
    qkv = x @ w_qkv ; q,k -> RoPE ; causal attention ; out = ctx @ w_proj

Sharding: tensor-parallel over heads across 8 cores (2 heads/core).
Each core reads the full x (transposed + bf16-cast on host), its slice of
w_qkv/w_proj, computes attention for its 2 heads and a *partial*
projection output; the host sums the 8 partials (the gather step of
row-parallel TP, replacing the all-reduce).

Per-core layout choices:
  - all data-path tensors are bf16 (PSUM accumulation stays fp32); the
    2e-2 rel-err budget dwarfs bf16 rounding, and bf16 halves DMA bytes,
    doubles/quadruples DVE throughput and enables fast weight loads.
  - x is fed transposed (d on partitions) so the QKV projection produces
    qT/kT directly in (feature, token) layout for the scores matmul.
  - scores are computed transposed (keys on partitions, queries free):
    exp runs on ACT along the free dim; the softmax denominator comes
    from 64 ones-columns in the v stationary operand, which broadcast
    the denominator across partitions 64:128 of the PV accumulator.
  - RoPE's rotate-half runs as a PE permutation matmul (rotm), with the
    sin table pre-permuted/sign-folded on the host.
  - causal masking: blocks strictly below the diagonal are unmasked;
    band blocks get a 128x128 triangular 0/1 mask multiply and their
    fully-masked column prefix is skipped entirely (matmul, exp and PV
    are column-trimmed).
  - batches are software-pipelined: batch 1's QKV projection work units
    are injected between batch 0's attention iterations to fill the PE
    during exp waits; PV matmuls are deferred by one iteration.
"""

import functools
import os
import sys

import numpy as np

sys.path.insert(0, "/opt/trn_rl_repo")

# ---- problem constants (must match reference.py) ----
B = 2
S = 2048
D = 1024
H = 16
Dh = 64
N_CORES = 8
HPC = H // N_CORES          # heads per core = 2
KC = D // 128               # contraction chunks = 8
TCH = 512                   # token chunk for stage 1
NTCH = S // TCH             # 4 chunks per batch
NSUB = S // 128             # 16 key subchunks per batch
ROPE_BASE = 10000.0
SCALE = 1.0 / 8.0           # 1/sqrt(Dh)


def _build_program(loop_n=1, phases="all", opts=""):
    import concourse.bass as bass  # noqa: F401
    opts = set(opts.split(",")) if opts else set()
    import concourse.mybir as mybir
    import concourse.tile as tile
    from concourse import bacc
    from contextlib import ExitStack

    FP = mybir.dt.float32
    BF = mybir.dt.bfloat16
    EXP = mybir.ActivationFunctionType.Exp

    nc = bacc.Bacc("TRN2", target_bir_lowering=False, debug=False)

    xt_d = nc.dram_tensor("xt", [B, KC, 128, S], BF, kind="ExternalInput").ap()
    wqk_d = nc.dram_tensor("wqk", [KC, 128, 3 * 128], BF, kind="ExternalInput").ap()
    wproj_d = nc.dram_tensor("wproj", [128, D], BF, kind="ExternalInput").ap()
    cos_d = nc.dram_tensor("cost", [128, S], BF, kind="ExternalInput").ap()
    sin_d = nc.dram_tensor("sint", [128, S], BF, kind="ExternalInput").ap()
    tri_d = nc.dram_tensor("tri", [128, HPC, 128], BF, kind="ExternalInput").ap()
    ones_d = nc.dram_tensor("onesc", [128, NSUB, HPC, 64], BF, kind="ExternalInput").ap()
    ident_d = nc.dram_tensor("ident", [128, 128], BF, kind="ExternalInput").ap()
    rotm_d = nc.dram_tensor("rotm", [128, 128], BF, kind="ExternalInput").ap()
    out_d = nc.dram_tensor("out", [B, S, D], BF, kind="ExternalOutput").ap()

    with tile.TileContext(nc) as tc, ExitStack() as ctx:
        consts = ctx.enter_context(tc.tile_pool(name="consts", bufs=1))
        store = ctx.enter_context(tc.tile_pool(name="store", bufs=1))
        xt_pool = ctx.enter_context(tc.tile_pool(name="xt_pool", bufs=4))
        rope_pool = ctx.enter_context(tc.tile_pool(name="rope_pool", bufs=3))
        p_pool = ctx.enter_context(tc.tile_pool(name="p_pool", bufs=8))
        nrm_pool = ctx.enter_context(tc.tile_pool(name="nrm_pool", bufs=3))
        ob_pool = ctx.enter_context(tc.tile_pool(name="ob_pool", bufs=4))
        ps_qk = ctx.enter_context(tc.tile_pool(name="ps_qk", bufs=2, space="PSUM"))
        ps_s = ctx.enter_context(tc.tile_pool(name="ps_s", bufs=2, space="PSUM"))
        ps_pv = ctx.enter_context(tc.tile_pool(name="ps_pv", bufs=2, space="PSUM"))
        ps_vo = ctx.enter_context(tc.tile_pool(name="ps_vo", bufs=2, space="PSUM"))

        # ---- constants: only the fc0 weight chunk blocks the first
        # matmuls; everything else is ordered by first use.
        wqk_sb = consts.tile([128, KC, 384], BF, name="wqk_sb")
        nc.sync.dma_start(
            wqk_sb[:, :, 0:128],
            wqk_d.rearrange("k p f -> p k f")[:, :, 0:128],
        )
        ident_sb = consts.tile([128, 128], BF, name="ident_sb")
        rotm_sb = consts.tile([128, 128], BF, name="rotm_sb")
        wproj_sb = consts.tile([128, D], BF, name="wproj_sb")
        cos_sb = consts.tile([128, S], BF, name="cos_sb")
        sin_sb = consts.tile([128, S], BF, name="sin_sb")
        tri_sb = consts.tile([128, HPC, 128], BF, name="tri_sb")

        def late_consts():
            # ordered by first use: wqk fc1 gates the 2nd matmul group,
            # rotm/sin/cos gate the first rope drain.
            nc.sync.dma_start(
                wqk_sb[:, :, 128:256],
                wqk_d.rearrange("k p f -> p k f")[:, :, 128:256],
            )
            nc.sync.dma_start(rotm_sb, rotm_d)
            nc.sync.dma_start(sin_sb, sin_d)
            nc.sync.dma_start(cos_sb, cos_d)
            nc.sync.dma_start(
                wqk_sb[:, :, 256:384],
                wqk_d.rearrange("k p f -> p k f")[:, :, 256:384],
            )
            nc.sync.dma_start(ident_sb, ident_d)

        def later_consts():
            for bb in range(B):
                nc.sync.dma_start(vt[bb][:, :, :, 64:128], ones_d)
            nc.sync.dma_start(tri_sb, tri_d)
            nc.sync.dma_start(wproj_sb, wproj_d)

        # ---- persistent per-batch storage ----
        qT = {}   # (b, t) -> (128, 512)  rows: [h0 d0..63 | h1 d0..63]
        kT = {}
        vt = {}   # b -> (128, NSUB, HPC, 128) cols: [v_h | ones] per subchunk
        ctxT = {}
        for b in range(B):
            ctxT[b] = store.tile([128, S], BF, name=f"ctxT_{b}", tag=f"ctxT_{b}")
            for t in range(NTCH):
                qT[b, t] = store.tile([128, TCH], BF, name=f"qT_{b}_{t}", tag=f"qT_{b}_{t}")
                kT[b, t] = store.tile([128, TCH], BF, name=f"kT_{b}_{t}", tag=f"kT_{b}_{t}")
            vt[b] = store.tile([128, NSUB, HPC, 128], BF, name=f"vt_{b}", tag=f"vt_{b}")

        def stage1_units(b, first=False):
            """QKV^T projection + RoPE + v natural layout, for batch b,
            as a list of work-unit closures for interleaving with attention.

            t-chunks are processed in pairs so each weight chunk loaded into
            the PE serves two consecutive matmuls."""
            npair = 2
            loads = []
            mms = []
            for tp in range(NTCH // npair):
                xts = {}

                def u_load(b=b, tp=tp, xts=xts):
                    for i in range(npair):
                        xts[i] = xt_pool.tile([128, KC, TCH], BF, name="xtile", tag="xt")
                    # x loads go through the idle Pool engine's SWDGE queue so
                    # they run on a different DGE ring than the SP-issued
                    # constants.  Only the first pair is split into kc-halves
                    # (so its matmuls start after half a tile); later pairs
                    # are prefetched far ahead and load whole.
                    halves = (0, KC // 2) if (first and tp == 0) else (0,)
                    slab = KC // len(halves) if len(halves) > 1 else KC
                    for kc0 in halves:
                        for i in range(npair):
                            t = npair * tp + i
                            tsl = slice(t * TCH, (t + 1) * TCH)
                            nc.gpsimd.dma_start(
                                xts[i][:, kc0:kc0 + slab, :],
                                xt_d[b].rearrange("k p f -> p k f")[
                                    :, kc0:kc0 + slab, tsl
                                ],
                            )
                            if first and tp == 0 and kc0 == 0 and i == 0:
                                late_consts()
                    if first and tp == 1:
                        later_consts()

                loads.append(u_load)

                # --- q, k (transposed, RoPE) and vT ---
                for fc in range(3):
                    def u_mm(b=b, tp=tp, fc=fc, xts=xts):
                        psq = {}
                        for i in range(npair):
                            psq[i] = ps_qk.tile([128, TCH], FP, name="pqk", tag="ps_qk")
                        for kc in range(KC):
                            for i in range(npair):
                                nc.tensor.matmul(
                                    psq[i],
                                    lhsT=wqk_sb[:, kc, fc * 128:(fc + 1) * 128],
                                    rhs=xts[i][:, kc, :],
                                    start=(kc == 0),
                                    stop=(kc == KC - 1),
                                )
                        for i in range(npair):
                            t = npair * tp + i
                            tsl = slice(t * TCH, (t + 1) * TCH)
                            _stage1_drain(b, t, tsl, fc, psq[i])

                    mms.append(u_mm)
            if first:
                # issue every x load up front: they ride the Pool DGE ring in
                # parallel with the SP-issued constants, so deeper prefetch
                # shortens the DMA-starved startup.
                return loads + mms
            # as attention filler, keep loads just ahead of their consumers.
            k = len(mms) // len(loads)
            units = []
            for i, ld in enumerate(loads):
                units.append(ld)
                units.extend(mms[i * k:(i + 1) * k])
            return units

        def _stage1_drain(b, t, tsl, fc, pqk):
            if fc < 2:
                # RoPE: dest = q*cos + rot(q)*sin, with the partition rotation
                # done by a PE permutation matmul (rotm) instead of DMAs.
                dest = qT[b, t] if fc == 0 else kT[b, t]
                s1 = rope_pool.tile([128, TCH], BF, name="rs1", tag="rs1")
                nc.vector.tensor_mul(out=s1, in0=pqk, in1=sin_sb[:, tsl])
                s2 = rope_pool.tile([128, TCH], BF, name="rs2", tag="rs2")
                nc.vector.tensor_mul(out=s2, in0=pqk, in1=cos_sb[:, tsl])
                pr = ps_vo.tile([128, TCH], FP, name="prot", tag="ps_vo")
                nc.tensor.matmul(pr, lhsT=rotm_sb, rhs=s1, start=True, stop=True)
                nc.vector.tensor_add(out=dest, in0=pr, in1=s2)
            else:
                # vT -> transpose to natural layout via PE
                vts = rope_pool.tile([128, TCH], BF, name="vts", tag="vts")
                nc.scalar.copy(vts, pqk)
                pv4 = ps_vo.tile([128, 4, 128], BF, name="pv4", tag="ps_vo")
                for sc4 in range(4):
                    nc.tensor.transpose(
                        pv4[:, sc4, :], vts[:, sc4 * 128:(sc4 + 1) * 128],
                        ident_sb,
                    )
                # pv4 cols [0:64]=h0 feats, [64:128]=h1 feats -> vt[..., h, 0:64]
                v2 = vt[b][:, t * 4:(t + 1) * 4, :, 0:64]
                s2 = pv4.rearrange("p s (h c) -> p s h c", c=64)
                nc.vector.tensor_copy(out=v2, in_=s2)

        def _pv_mm(b, pv, nkc, kc, pp):
            off = max(0, (kc - (nkc - 4)) * 128)
            for h in range(HPC):
                nc.tensor.matmul(
                    pv[:, h, off:TCH],
                    lhsT=vt[b][:, kc, h, :],
                    rhs=pp[:, h, off:TCH],
                    start=(kc == 0),
                    stop=(kc == nkc - 1),
                )

        def proj_qb_units(b, qb, last=False):
            """Projection of one 512-token qb as 4 work units (one per
            128-token block; 2 matmuls + drains each) sharing one ob tile,
            written out as a single DMA.  `last` keeps per-block DMAs so the
            kernel tail stays pipelined."""
            ob = {}

            def mk(tbi, tb):
                def u():
                    if tbi == 0:
                        ob[0] = ob_pool.tile([128, 4, D], BF, name="ob4", tag="ob")
                    for ec in range(2):
                        po = ps_vo.tile([128, 512], FP, name="po", tag="ps_vo")
                        nc.tensor.matmul(
                            po,
                            lhsT=ctxT[b][:, tb * 128:(tb + 1) * 128],
                            rhs=wproj_sb[:, ec * 512:(ec + 1) * 512],
                            start=True,
                            stop=True,
                        )
                        obs = ob[0][:, tbi, ec * 512:(ec + 1) * 512]
                        if ec == 0:
                            nc.scalar.copy(obs, po)
                        else:
                            nc.vector.tensor_copy(out=obs, in_=po)
                    if last:
                        nc.sync.dma_start(
                            out_d[b, tb * 128:(tb + 1) * 128, :], ob[0][:, tbi, :]
                        )
                    elif tbi == 3:
                        nc.sync.dma_start(
                            out_d[b, qb * TCH:(qb + 1) * TCH, :].rearrange(
                                "(blk p) d -> p blk d", p=128
                            ),
                            ob[0],
                        )

                return u

            return [mk(tbi, tb) for tbi, tb in enumerate(range(4 * qb, 4 * qb + 4))]

        def attention(b, filler=(), emit_proj=True):
            """Causal attention for both heads of batch b -> normalized ctxT.

            PV matmuls are deferred by one kc iteration so the exp feeding
            them has a full score-slot of latency headroom before the PE
            queue reaches the PV.  `filler` is a list of independent
            work-unit closures (the next batch's stage1, or the previous
            batch's projection) injected between kc iterations to keep the
            PE busy during exp waits.  With emit_proj=False this batch's
            projection is left to the caller (to use as later filler)."""
            filler = list(filler)
            total_iters = sum(4 * qb + 4 for qb in range(NTCH))
            stride = max(1, total_iters // len(filler)) if filler else 0
            fill_i = 0
            iter_i = 0

            def maybe_fill():
                nonlocal fill_i
                if filler and iter_i % stride == 0 and fill_i < len(filler):
                    filler[fill_i]()
                    fill_i += 1

            for qb in range(NTCH):
                qsl = slice(qb * TCH, (qb + 1) * TCH)
                nkc = 4 * qb + 4
                # both heads' accumulators in one 2-bank tile: rows 64:128 of
                # each bank hold that head's denominator.
                pv = ps_pv.tile([128, HPC, TCH], FP, name="ppv", tag="ps_pv", bufs=1)
                pending = []
                for kc in range(nkc):
                    off = max(0, (kc - 4 * qb) * 128)
                    nv = TCH - off
                    # scores for both heads back-to-back: h0 uses PE rows 0-63,
                    # h1 rows 64-127, so the two matmuls run concurrently; the
                    # exp outputs share one 2-head p tile.
                    p = p_pool.tile([128, HPC, TCH], BF, name="p", tag="p")
                    for h in range(HPC):
                        hb = h * 64
                        ps = ps_s.tile([128, TCH], FP, name="ps", tag="ps_s")
                        nc.tensor.matmul(
                            ps[:, :nv],
                            lhsT=kT[b, kc // 4][hb:hb + 64, (kc % 4) * 128:(kc % 4 + 1) * 128],
                            rhs=qT[b, qb][hb:hb + 64, off:TCH],
                            start=True,
                            stop=True,
                        )
                        nc.scalar.activation(p[:, h, off:TCH], ps[:, :nv], EXP, scale=SCALE)
                    if kc >= 4 * qb:  # diagonal band: one triangular mask
                        nc.vector.tensor_mul(
                            out=p[:, :, off:off + 128],
                            in0=p[:, :, off:off + 128],
                            in1=tri_sb,
                        )
                    pending.append((kc, p))
                    if len(pending) > 1:
                        _pv_mm(b, pv, nkc, *pending.pop(0))
                    iter_i += 1
                    maybe_fill()
                for item in pending:
                    _pv_mm(b, pv, nkc, *item)
                # normalize: ctx rows 0..63; rows 64..127 hold the denominator
                # (broadcast across partitions by the ones columns of vt).
                # One reciprocal covers both heads.
                rcb = nrm_pool.tile([64, HPC, TCH], FP, name="rcb", tag="rcb")
                nc.vector.reciprocal(rcb, pv[64:128, :, :])
                # h1 first: its partition-shifting DMA overlaps h0's multiply.
                ctmp = nrm_pool.tile([64, TCH], BF, name="ctmp", tag="ctmp")
                nc.vector.tensor_mul(out=ctmp, in0=pv[0:64, 1, :], in1=rcb[:, 1, :])
                nc.gpsimd.dma_start(ctxT[b][64:128, qsl], ctmp)
                nc.vector.tensor_mul(
                    out=ctxT[b][0:64, qsl], in0=pv[0:64, 0, :], in1=rcb[:, 0, :]
                )
                if emit_proj:
                    for u in proj_qb_units(b, qb, last=(qb == NTCH - 1)):
                        u()

        def whole():
            for u in stage1_units(0, first=True):
                u()
            if phases == "s1":
                for u in stage1_units(1):
                    u()
                return
            attention(0, filler=stage1_units(1), emit_proj=False)
            attention(
                1,
                filler=[u for qb in range(NTCH) for u in proj_qb_units(0, qb)],
            )

        if loop_n == 1:
            whole()
        else:
            with tc.For_i(0, loop_n, 1):
                whole()

    nc.compile()
    return nc


@functools.lru_cache(maxsize=4)
def _get_program(loop_n=1, phases="all", opts=""):
    return _build_program(loop_n, phases, opts)


def _host_inputs(x, w_qkv, w_proj):
    """Build the 8 per-core input maps from the full problem inputs."""
    from ml_dtypes import bfloat16

    x = np.asarray(x, dtype=np.float32)
    w_qkv = np.asarray(w_qkv, dtype=np.float32)
    w_proj = np.asarray(w_proj, dtype=np.float32)

    xt = np.ascontiguousarray(x.transpose(0, 2, 1)).reshape(B, KC, 128, S)
    xt = xt.astype(bfloat16)

    # RoPE tables, transposed + head-replicated; sin is sign-folded.
    inv_freq = 1.0 / (ROPE_BASE ** (np.arange(0, Dh, 2, dtype=np.float32) / Dh))
    tpos = np.arange(S, dtype=np.float32)
    freqs = np.outer(tpos, inv_freq)                      # (S, 32)
    emb = np.concatenate([freqs, freqs], axis=-1)         # (S, 64)
    cosT = np.cos(emb).T.astype(np.float32)               # (64, S)
    sinT = np.sin(emb).T.astype(np.float32)
    sinT_f = sinT.copy()
    sinT_f[:32] *= -1.0                                   # fold rotate_half sign
    cos_full = np.tile(cosT, (2, 1)).astype(bfloat16)     # (128, S)
    sin_full = np.tile(sinT_f, (2, 1)).astype(bfloat16)   # (128, S)
    # sin multiplies BEFORE the XOR-32 rotation matmul: permute rows so the
    # folded sign/value land on the right partition after rotation.
    sin_full = np.ascontiguousarray(sin_full[np.arange(128) ^ 32])

    r = np.arange(128)
    tri1 = (r[None, :] >= r[:, None]).astype(bfloat16)    # tri[r, c] = c >= r
    tri = np.ascontiguousarray(
        np.broadcast_to(tri1[:, None, :], (128, HPC, 128))
    )
    # XOR-32 partition permutation (rotate_half); sin sign is folded in sint.
    rotm = np.zeros((128, 128), dtype=bfloat16)
    rotm[r, r ^ 32] = 1.0

    wq = w_qkv[:, 0:D]
    wk = w_qkv[:, D:2 * D]
    wv = w_qkv[:, 2 * D:3 * D]

    in_maps = []
    for c in range(N_CORES):
        h0, h1 = 2 * c, 2 * c + 1
        cols = np.r_[h0 * 64:(h0 + 1) * 64, h1 * 64:(h1 + 1) * 64]
        wqk_c = np.concatenate([wq[:, cols], wk[:, cols], wv[:, cols]], axis=1)  # (D, 384)
        in_maps.append({
            "xt": xt,
            "wqk": np.ascontiguousarray(wqk_c).reshape(KC, 128, 384).astype(bfloat16),
            "wproj": np.ascontiguousarray(w_proj[c * 128:(c + 1) * 128, :]).astype(bfloat16),
            "cost": cos_full,
            "sint": sin_full,
            "tri": tri,
            "onesc": np.ones((128, NSUB, HPC, 64), dtype=bfloat16),
            "ident": np.eye(128, dtype=bfloat16),
            "rotm": rotm,
        })
    return in_maps


_last_results = None


def kernel(x, w_qkv, w_proj):
    global _last_results
    from concourse.bass_utils import run_bass_kernel_spmd

    nc = _get_program()
    in_maps = _host_inputs(x, w_qkv, w_proj)
    trace = bool(int(os.environ.get("KERNEL_TRACE", "0")))
    kwargs = {}
    if trace:
        kwargs["trace"] = True
        kwargs["trace_cores"] = list(range(N_CORES))
    res = run_bass_kernel_spmd(nc, in_maps, core_ids=list(range(N_CORES)), **kwargs)
    _last_results = res
    acc = np.zeros((B, S, D), dtype=np.float64)
    for r in res.results:
        acc += np.asarray(r["out"], dtype=np.float64)
    return acc.astype(np.float32)
